# revision 9
# baseline (speedup 1.0000x reference)
"""Trainium2 Bass kernel for nn_Attention_70660801954538.

Multi-head attention with an affinity gate and two auxiliary outputs:
    q = xq @ Wq.T ; k = xk @ Wk.T ; v = xv @ Wv.T          (per batch item)
    attn      = (q_h @ k_h.T) * SCALE   per head           [B,H,N,N]
    aff_mask  = sigmoid(mean_h (et_h . k_h) * SCALE)       [B,1,1,N]
    attn_save = mean_h attn                                 [B,N,N]
    x = (aff_mask * softmax(attn)) @ v  -> concat -> @ Wp.T + bp

Sharding: pure data-parallel — one batch item per NeuronCore (B=8, 8 cores),
no collectives.

Key algebraic restructurings (per core):
  * attn_save = (q @ k.T over the FULL C dim) * SCALE/H — one dense matmul
    instead of a 12-way sum of per-head score matrices.
  * aff_mask folds into v (column scaling) instead of scaling the NxN
    probability matrices.
  * softmax denominator Z comes for free from an all-ones column appended to
    each head's v block (row 64 of the AV matmul output).
  * activations/weights are fed to the PE in bf16 (fp32 PSUM accumulation);
    all matmuls then run at the full 1 row/cycle rate.

Layouts: activations are pre-transposed on the host (xT layout [C, N]) so
that every matmul contraction has its index on the partition axis with no
on-chip transposes anywhere.
"""

import numpy as np

B, N, C, H = 8, 1024, 768, 12
HD = C // H           # 64
SCALE = HD ** -0.5    # 0.125
P = 128               # partitions
CK = C // P           # 6 c-chunks of 128
NT = N // P           # 8 n-tiles of 128
HW = HD + 1           # head slot width in v_aug (64 v columns + ones column)

_CACHE: dict = {}


def _build(repeats: int = 1, level: int = 7):
    import concourse.bacc as bacc
    import concourse.mybir as mybir
    from concourse.tile import TileContext

    f32 = mybir.dt.float32
    bf16 = mybir.dt.bfloat16
    Exp = mybir.ActivationFunctionType.Exp

    nc = bacc.Bacc(None, target_bir_lowering=False)

    xqT_d = nc.declare_dram_parameter("xqT", [C, N], bf16, isOutput=False)
    xkT_d = nc.declare_dram_parameter("xkT", [C, N], bf16, isOutput=False)
    xvT_d = nc.declare_dram_parameter("xvT", [C, N], bf16, isOutput=False)
    wqT_d = nc.declare_dram_parameter("wqT", [C, C], bf16, isOutput=False)
    wkT_d = nc.declare_dram_parameter("wkT", [C, C], bf16, isOutput=False)
    wvT_d = nc.declare_dram_parameter("wvT", [C, C], bf16, isOutput=False)
    wpT_d = nc.declare_dram_parameter("wpT", [C, C], bf16, isOutput=False)
    uc_d = nc.declare_dram_parameter("uc", [P, CK], bf16, isOutput=False)
    bp_d = nc.declare_dram_parameter("bp", [1, C], bf16, isOutput=False)
    ox_d = nc.declare_dram_parameter("out_x", [N, C], f32, isOutput=True)
    oa_d = nc.declare_dram_parameter("out_attn", [N, N], f32, isOutput=True)
    of_d = nc.declare_dram_parameter("out_aff", [1, N], f32, isOutput=True)

    xq_r = xqT_d[:, :].rearrange("(ck p) n -> p ck n", p=P)
    xk_r = xkT_d[:, :].rearrange("(ck p) n -> p ck n", p=P)
    xv_r = xvT_d[:, :].rearrange("(ck p) n -> p ck n", p=P)
    wq_r = wqT_d[:, :].rearrange("(ck p) c -> p ck c", p=P)
    wk_r = wkT_d[:, :].rearrange("(ck p) c -> p ck c", p=P)
    wv_r = wvT_d[:, :].rearrange("(ck p) c -> p ck c", p=P)
    wp_r = wpT_d[:, :].rearrange("(ck p) c -> p ck c", p=P)

    with TileContext(nc) as tc:
        with (
            tc.tile_pool(name="per", bufs=1) as per,
            tc.tile_pool(name="stg", bufs=2) as stg,
            tc.tile_pool(name="esp", bufs=24) as esp,
            tc.tile_pool(name="xin", bufs=2) as xin,
            tc.tile_pool(name="win", bufs=2) as win,
            tc.tile_pool(name="ps", bufs=2, space="PSUM") as ps,
        ):
            # ---- persistent tiles -------------------------------------
            qT = per.tile([P, CK, N], bf16)           # q^T  [c, n]
            kT = per.tile([P, CK, N], bf16)           # k^T  [c, n]
            v_aug = per.tile([P, NT, H * HW], bf16)   # [n, head-major v|1]
            xT = per.tile([P, CK, N], bf16)           # attention out^T
            wpT = per.tile([P, CK, C], bf16)
            ones_row = per.tile([P, P], bf16)
            bias_b = per.tile([P, C], f32)            # bp broadcast to rows
            uc = per.tile([P, CK], bf16)              # Wk.T @ extra_token, columns
            bp_r = per.tile([1, C], bf16)
            aff_row = per.tile([1, N], f32)
            aff_cols = per.tile([P, NT], f32)
            erow = per.tile([1, N], f32)

            nc.vector.memset(ones_row[:], 1.0)
            # ones column of every head slot in v_aug
            v4 = v_aug[:, :, :].rearrange("p t (h w) -> p t h w", w=HW)
            nc.vector.memset(v4[:, :, :, HD : HD + 1], 1.0)

            for rep in range(repeats):
                # ---- k projection (inputs first -- on the critical path)
                xk_t = xin.tile([P, CK, N], bf16, tag="x", name="xk_t")
                nc.sync.dma_start(out=xk_t[:], in_=xk_r)
                wk_t = win.tile([P, CK, C], bf16, tag="w", name="wk_t")
                nc.sync.dma_start(out=wk_t[:], in_=wk_r)
                xv_t = xin.tile([P, CK, N], bf16, tag="x", name="xv_t")
                nc.sync.dma_start(out=xv_t[:], in_=xv_r)
                wv_t = win.tile([P, CK, C], bf16, tag="w", name="wv_t")
                nc.sync.dma_start(out=wv_t[:], in_=wv_r)

                if rep == 0:
                    # static loads + bias broadcast rows: ones(128,1).T x bp
                    nc.sync.dma_start(out=wpT[:], in_=wp_r)
                    nc.sync.dma_start(out=uc[:], in_=uc_d[:, :])
                    nc.sync.dma_start(out=bp_r[:], in_=bp_d[:, :])
                    psb = ps.tile([P, 512], f32, tag="pj", name="psb")
                    nc.tensor.matmul(psb[:], ones_row[0:1, :], bp_r[:, 0:512])
                    nc.any.tensor_copy(out=bias_b[:, 0:512], in_=psb[:])
                    psb2 = ps.tile([P, 512], f32, tag="pj", name="psb2")
                    nc.tensor.matmul(psb2[:, 0:256], ones_row[0:1, :], bp_r[:, 512:C])
                    nc.any.tensor_copy(out=bias_b[:, 512:C], in_=psb2[:, 0:256])

                # ---- affinity gate row (u = Wk.T et, contracted with xk)
                for nh in range(2) if level >= 2 else []:
                    psa = ps.tile([1, 512], f32, tag="pj", name="psa")
                    for ck in range(CK):
                        nc.tensor.matmul(
                            psa[:],
                            uc[:, ck : ck + 1],
                            xk_t[:, ck, nh * 512 : (nh + 1) * 512],
                            start=(ck == 0),
                            stop=(ck == CK - 1),
                        )
                    # sigmoid(z) = 1/(1+exp(-z)); z = psa * SCALE/H
                    nc.scalar.activation(
                        out=erow[:, nh * 512 : (nh + 1) * 512],
                        in_=psa[:],
                        func=Exp,
                        scale=-SCALE / H,
                    )
                if level >= 2:
                    nc.vector.tensor_scalar_add(out=erow[:], in0=erow[:], scalar1=1.0)
                    nc.vector.reciprocal(out=aff_row[:], in_=erow[:])
                    nc.sync.dma_start(out=of_d[:, :], in_=aff_row[:])
                    # row -> per-n-tile columns via tiny PE outer products
                    afb = stg.tile([1, N], bf16, tag="afb", bufs=1, name="afb")
                    nc.any.tensor_copy(out=afb[:], in_=aff_row[:])
                    for nt in range(NT):
                        psc = ps.tile([P, 1], f32, tag="pj", name="psc")
                        nc.tensor.matmul(
                            psc[:], afb[0:1, nt * P : (nt + 1) * P], ones_row[0:1, 0:1]
                        )
                        nc.vector.tensor_copy(out=aff_cols[:, nt : nt + 1], in_=psc[:])

                # ---- v projection (+ aff scaling into v_aug) ----------
                for nt in range(NT) if level >= 3 else []:
                    psv = ps.tile([P, C], f32, tag="av", bufs=1, name="psv")
                    for ck in range(CK):
                        nc.tensor.matmul(
                            psv[:, 0:512],
                            xv_t[:, ck, nt * P : (nt + 1) * P],
                            wv_t[:, ck, 0:512],
                            start=(ck == 0),
                            stop=(ck == CK - 1),
                        )
                        nc.tensor.matmul(
                            psv[:, 512:C],
                            xv_t[:, ck, nt * P : (nt + 1) * P],
                            wv_t[:, ck, 512:C],
                            start=(ck == 0),
                            stop=(ck == CK - 1),
                        )
                    nc.vector.tensor_scalar_mul(
                        out=v4[:, nt, :, 0:HD],
                        in0=psv[:, :].rearrange("p (h d) -> p h d", d=HD),
                        scalar1=aff_cols[:, nt : nt + 1],
                    )

                # ---- k/q projections interleaved with scores+exp+AV ---
                xq_t = xin.tile([P, CK, N], bf16, tag="x", name="xq_t")
                nc.sync.dma_start(out=xq_t[:], in_=xq_r)
                wq_t = win.tile([P, CK, C], bf16, tag="w", name="wq_t")
                nc.sync.dma_start(out=wq_t[:], in_=wq_r)

                es_tiles = {}

                def emit_scores_pair(pr):
                    """Scores + exp for heads (2*pr, 2*pr+1); row-group packed."""
                    for kt in range(NT):
                        pss = {}
                        for hp in range(2):
                            pss[hp] = ps.tile([P, N], f32, tag="sc", name="pss")
                        for nh in range(2):
                            for hp in range(2):
                                base = hp * HD
                                nc.tensor.matmul(
                                    pss[hp][:, nh * 512 : (nh + 1) * 512],
                                    kT[base : base + HD, pr, kt * P : (kt + 1) * P],
                                    qT[base : base + HD, pr, nh * 512 : (nh + 1) * 512],
                                )
                        for hp in range(2):
                            est = esp.tile([P, N], bf16, tag="es", name="est")
                            nc.scalar.activation(
                                out=est[:], in_=pss[hp][:], func=Exp, scale=SCALE
                            )
                            es_tiles[(2 * pr + hp, kt)] = est

                def emit_av(h):
                    """p@v for head h: out^T rows 0..63, softmax Z in row 64."""
                    ch, hp = divmod(h, 2)
                    psav = ps.tile([HW, N], f32, tag="av", bufs=1, name="psav")
                    for nh in range(2):
                        for kc in range(NT):
                            nc.tensor.matmul(
                                psav[:, nh * 512 : (nh + 1) * 512],
                                v_aug[:, kc, h * HW : (h + 1) * HW],
                                es_tiles[(h, kc)][:, nh * 512 : (nh + 1) * 512],
                                start=(kc == 0),
                                stop=(kc == NT - 1),
                            )
                    zrow = stg.tile([HD + 1, N], f32, tag="zrow", bufs=1, name="zrow")
                    nc.vector.reciprocal(
                        out=zrow[HD : HD + 1, :], in_=psav[HD : HD + 1, :]
                    )
                    # partition_broadcast needs its source at partition 0 on HW;
                    # rebase with a partition-shifting SBUF->SBUF DMA first
                    zrow0 = stg.tile([1, N], f32, tag="zrow0", bufs=1, name="zrow0")
                    nc.sync.dma_start(out=zrow0[:], in_=zrow[HD : HD + 1, :])
                    zb = stg.tile([P, N], f32, tag="zb", bufs=1, name="zb")
                    nc.gpsimd.partition_broadcast(zb[:], zrow0[0:1, :])
                    if hp == 0:
                        nc.vector.tensor_mul(
                            out=xT[0:HD, ch, :], in0=psav[0:HD, :], in1=zb[0:HD, :]
                        )
                    else:
                        xodd = stg.tile([HD, N], bf16, tag="xodd", name="xodd")
                        nc.vector.tensor_mul(
                            out=xodd[:], in0=psav[0:HD, :], in1=zb[0:HD, :]
                        )
                        nc.sync.dma_start(out=xT[HD:P, ch, :], in_=xodd[:])

                for mo in range(CK):
                    for nh in range(2):
                        psk = ps.tile([P, 512], f32, tag="pj", name="psk")
                        for ki in range(CK):
                            nc.tensor.matmul(
                                psk[:],
                                wk_t[:, ki, mo * P : (mo + 1) * P],
                                xk_t[:, ki, nh * 512 : (nh + 1) * 512],
                                start=(ki == 0),
                                stop=(ki == CK - 1),
                            )
                        nc.vector.tensor_copy(
                            out=kT[:, mo, nh * 512 : (nh + 1) * 512], in_=psk[:]
                        )
                    if level < 4:
                        continue
                    for nh in range(2):
                        psq = ps.tile([P, 512], f32, tag="pj", name="psq")
                        for ki in range(CK):
                            nc.tensor.matmul(
                                psq[:],
                                wq_t[:, ki, mo * P : (mo + 1) * P],
                                xq_t[:, ki, nh * 512 : (nh + 1) * 512],
                                start=(ki == 0),
                                stop=(ki == CK - 1),
                            )
                        nc.vector.tensor_copy(
                            out=qT[:, mo, nh * 512 : (nh + 1) * 512], in_=psq[:]
                        )
                    emit_scores_pair(mo)
                    if mo >= 1 and level >= 5:
                        emit_av(2 * (mo - 1))
                        emit_av(2 * (mo - 1) + 1)

                # ---- attn_save (full-C contraction) + remaining AV ----
                def emit_attn_tile(qt):
                    psat = ps.tile([P, N], f32, tag="sc", name="psat")
                    for ck in range(CK):
                        for nh in range(2):
                            nc.tensor.matmul(
                                psat[:, nh * 512 : (nh + 1) * 512],
                                qT[:, ck, qt * P : (qt + 1) * P],
                                kT[:, ck, nh * 512 : (nh + 1) * 512],
                                start=(ck == 0),
                                stop=(ck == CK - 1),
                            )
                    stA = stg.tile([P, N], f32, tag="stA", name="stA")
                    nc.vector.tensor_scalar_mul(
                        out=stA[:], in0=psat[:], scalar1=SCALE / H
                    )
                    nc.sync.dma_start(out=oa_d[qt * P : (qt + 1) * P, :], in_=stA[:])

                if level >= 6:
                    emit_attn_tile(0)
                if level >= 5:
                    emit_av(10)
                if level >= 6:
                    emit_attn_tile(1)
                if level >= 5:
                    emit_av(11)
                if level >= 6:
                    for qt in range(2, NT):
                        emit_attn_tile(qt)

                # ---- output projection --------------------------------
                for qt in range(NT) if level >= 7 else []:
                    psy = ps.tile([P, C], f32, tag="sc", name="psy")
                    for c0, cw in ((0, 512), (512, 256)):
                        for ch in range(CK):
                            nc.tensor.matmul(
                                psy[:, c0 : c0 + cw],
                                xT[:, ch, qt * P : (qt + 1) * P],
                                wpT[:, ch, c0 : c0 + cw],
                                start=(ch == 0),
                                stop=(ch == CK - 1),
                            )
                    ystage = stg.tile([P, C], f32, tag="ystage", name="ystage")
                    nc.vector.tensor_add(out=ystage[:], in0=psy[:], in1=bias_b[:])
                    nc.sync.dma_start(
                        out=ox_d[qt * P : (qt + 1) * P, :], in_=ystage[:]
                    )

    nc.compile()
    return nc


def _get_nc(repeats: int = 1):
    key = repeats
    if key not in _CACHE:
        _CACHE[key] = _build(repeats)
    return _CACHE[key]


def _prep_inputs(xq, xk, xv, extra_token, Wq, Wk, Wv, Wp, bp):
    import ml_dtypes

    bf = ml_dtypes.bfloat16
    arrs = [np.asarray(a, dtype=np.float32) for a in (xq, xk, xv, extra_token)]
    xq, xk, xv, et = arrs
    wq_b = np.ascontiguousarray(np.asarray(Wq, np.float32).T).astype(bf)
    wk_b = np.ascontiguousarray(np.asarray(Wk, np.float32).T).astype(bf)
    wv_b = np.ascontiguousarray(np.asarray(Wv, np.float32).T).astype(bf)
    wp_b = np.ascontiguousarray(np.asarray(Wp, np.float32).T).astype(bf)
    bp_b = np.asarray(bp, np.float32).reshape(1, C).astype(bf)
    in_maps = []
    for b in range(B):
        in_maps.append(
            {
                "xqT": np.ascontiguousarray(xq[b].T).astype(bf),
                "xkT": np.ascontiguousarray(xk[b].T).astype(bf),
                "xvT": np.ascontiguousarray(xv[b].T).astype(bf),
                "wqT": wq_b,
                "wkT": wk_b,
                "wvT": wv_b,
                "wpT": wp_b,
                "etc": np.ascontiguousarray(et[b].reshape(CK, P).T).astype(bf),
                "bp": bp_b,
            }
        )
    return in_maps


def kernel(xq, xk, xv, extra_token, Wq, Wk, Wv, Wp, bp):
    from concourse.bass_utils import run_bass_kernel_spmd

    nc = _get_nc(1)
    in_maps = _prep_inputs(xq, xk, xv, extra_token, Wq, Wk, Wv, Wp, bp)
    r = run_bass_kernel_spmd(nc, in_maps, core_ids=list(range(B)))
    x = np.stack([r.results[i]["out_x"] for i in range(B)])
    attn = np.stack([r.results[i]["out_attn"] for i in range(B)])
    aff = np.stack([r.results[i]["out_aff"] for i in range(B)]).reshape(B, 1, 1, N)
    return (
        x.astype(np.float32),
        attn.astype(np.float32),
        aff.astype(np.float32),
    )


if __name__ == "__main__":
    import time

    t0 = time.time()
    nc = _build(1)
    print(f"build+compile(py-side): {time.time() - t0:.1f}s")
    from concourse.timeline_sim import TimelineSim

    t0 = time.time()
    sim = TimelineSim(nc)
    dur = sim.simulate()
    print(f"TimelineSim predicted: {dur:.0f} ns  (sim took {time.time() - t0:.1f}s)")


# revision 10
# speedup vs baseline: 8.3390x; 8.3390x over previous
"""Trainium2 Bass kernel for nn_Attention_70660801954538.

Multi-head attention with an affinity gate and two auxiliary outputs:
    q = xq @ Wq.T ; k = xk @ Wk.T ; v = xv @ Wv.T          (per batch item)
    attn      = (q_h @ k_h.T) * SCALE   per head           [B,H,N,N]
    aff_mask  = sigmoid(mean_h (et_h . k_h) * SCALE)       [B,1,1,N]
    attn_save = mean_h attn                                 [B,N,N]
    x = (aff_mask * softmax(attn)) @ v  -> concat -> @ Wp.T + bp

Sharding: pure data-parallel — one batch item per NeuronCore (B=8, 8 cores),
no collectives.

Key algebraic restructurings (per core):
  * attn_save = (q @ k.T over the FULL C dim) * SCALE/H — one dense matmul
    instead of a 12-way sum of per-head score matrices.
  * aff_mask folds into v (column scaling) instead of scaling the NxN
    probability matrices.
  * softmax denominator Z comes for free from an all-ones column appended to
    each head's v block (row 64 of the AV matmul output).
  * activations/weights are fed to the PE in bf16 (fp32 PSUM accumulation);
    all matmuls then run at the full 1 row/cycle rate.

Layouts: activations are pre-transposed on the host (xT layout [C, N]) so
that every matmul contraction has its index on the partition axis with no
on-chip transposes anywhere.
"""

import numpy as np

B, N, C, H = 8, 1024, 768, 12
HD = C // H           # 64
SCALE = HD ** -0.5    # 0.125
P = 128               # partitions
CK = C // P           # 6 c-chunks of 128
NT = N // P           # 8 n-tiles of 128
HW = HD + 1           # head slot width in v_aug (64 v columns + ones column)

_CACHE: dict = {}


def _build(repeats: int = 1, level: int = 7):
    import concourse.bacc as bacc
    import concourse.mybir as mybir
    from concourse.tile import TileContext

    f32 = mybir.dt.float32
    bf16 = mybir.dt.bfloat16
    Exp = mybir.ActivationFunctionType.Exp

    nc = bacc.Bacc(None, target_bir_lowering=False)

    xqT_d = nc.declare_dram_parameter("xqT", [C, N], bf16, isOutput=False)
    xkT_d = nc.declare_dram_parameter("xkT", [C, N], bf16, isOutput=False)
    xvT_d = nc.declare_dram_parameter("xvT", [C, N], bf16, isOutput=False)
    wqT_d = nc.declare_dram_parameter("wqT", [C, C], bf16, isOutput=False)
    wkT_d = nc.declare_dram_parameter("wkT", [C, C], bf16, isOutput=False)
    wvT_d = nc.declare_dram_parameter("wvT", [C, C], bf16, isOutput=False)
    wpT_d = nc.declare_dram_parameter("wpT", [C, C], bf16, isOutput=False)
    uc_d = nc.declare_dram_parameter("uc", [P, CK], bf16, isOutput=False)
    bp_d = nc.declare_dram_parameter("bp", [1, C], bf16, isOutput=False)
    ox_d = nc.declare_dram_parameter("out_x", [N, C], f32, isOutput=True)
    oa_d = nc.declare_dram_parameter("out_attn", [N, N], f32, isOutput=True)
    of_d = nc.declare_dram_parameter("out_aff", [1, N], f32, isOutput=True)

    xq_r = xqT_d[:, :].rearrange("(ck p) n -> p ck n", p=P)
    xk_r = xkT_d[:, :].rearrange("(ck p) n -> p ck n", p=P)
    xv_r = xvT_d[:, :].rearrange("(ck p) n -> p ck n", p=P)
    wq_r = wqT_d[:, :].rearrange("(ck p) c -> p ck c", p=P)
    wk_r = wkT_d[:, :].rearrange("(ck p) c -> p ck c", p=P)
    wv_r = wvT_d[:, :].rearrange("(ck p) c -> p ck c", p=P)
    wp_r = wpT_d[:, :].rearrange("(ck p) c -> p ck c", p=P)

    with TileContext(nc) as tc:
        with (
            tc.tile_pool(name="per", bufs=1) as per,
            tc.tile_pool(name="stg", bufs=2) as stg,
            tc.tile_pool(name="esp", bufs=24) as esp,
            tc.tile_pool(name="xin", bufs=2) as xin,
            tc.tile_pool(name="win", bufs=2) as win,
            tc.tile_pool(name="ps", bufs=2, space="PSUM") as ps,
        ):
            # ---- persistent tiles -------------------------------------
            qT = per.tile([P, CK, N], bf16)           # q^T  [c, n]
            kT = per.tile([P, CK, N], bf16)           # k^T  [c, n]
            v_aug = per.tile([P, NT, H * HW], bf16)   # [n, head-major v|1]
            xT = per.tile([P, CK, N], bf16)           # attention out^T
            wpT = per.tile([P, CK, C], bf16)
            ones_row = per.tile([P, P], bf16)
            bias_b = per.tile([P, C], f32)            # bp broadcast to rows
            uc = per.tile([P, CK], bf16)              # Wk.T @ extra_token, columns
            bp_r = per.tile([1, C], bf16)
            aff_row = per.tile([1, N], f32)
            aff_cols = per.tile([P, NT], f32)
            erow = per.tile([1, N], f32)

            nc.vector.memset(ones_row[:], 1.0)
            # ones column of every head slot in v_aug
            v4 = v_aug[:, :, :].rearrange("p t (h w) -> p t h w", w=HW)
            nc.vector.memset(v4[:, :, :, HD : HD + 1], 1.0)

            for rep in range(repeats):
                # ---- k projection (inputs first -- on the critical path)
                xk_t = xin.tile([P, CK, N], bf16, tag="x", name="xk_t")
                nc.sync.dma_start(out=xk_t[:], in_=xk_r)
                wk_t = win.tile([P, CK, C], bf16, tag="w", name="wk_t")
                nc.sync.dma_start(out=wk_t[:], in_=wk_r)
                xv_t = xin.tile([P, CK, N], bf16, tag="x", name="xv_t")
                nc.sync.dma_start(out=xv_t[:], in_=xv_r)
                wv_t = win.tile([P, CK, C], bf16, tag="w", name="wv_t")
                nc.sync.dma_start(out=wv_t[:], in_=wv_r)

                if rep == 0:
                    # static loads + bias broadcast rows: ones(128,1).T x bp
                    nc.sync.dma_start(out=wpT[:], in_=wp_r)
                    nc.sync.dma_start(out=uc[:], in_=uc_d[:, :])
                    nc.sync.dma_start(out=bp_r[:], in_=bp_d[:, :])
                    psb = ps.tile([P, 512], f32, tag="pj", name="psb")
                    nc.tensor.matmul(psb[:], ones_row[0:1, :], bp_r[:, 0:512])
                    nc.any.tensor_copy(out=bias_b[:, 0:512], in_=psb[:])
                    psb2 = ps.tile([P, 512], f32, tag="pj", name="psb2")
                    nc.tensor.matmul(psb2[:, 0:256], ones_row[0:1, :], bp_r[:, 512:C])
                    nc.any.tensor_copy(out=bias_b[:, 512:C], in_=psb2[:, 0:256])

                # ---- affinity gate row (u = Wk.T et, contracted with xk)
                for nh in range(2) if level >= 2 else []:
                    psa = ps.tile([1, 512], f32, tag="pj", name="psa")
                    for ck in range(CK):
                        nc.tensor.matmul(
                            psa[:],
                            uc[:, ck : ck + 1],
                            xk_t[:, ck, nh * 512 : (nh + 1) * 512],
                            start=(ck == 0),
                            stop=(ck == CK - 1),
                        )
                    # sigmoid(z) = 1/(1+exp(-z)); z = psa * SCALE/H
                    nc.scalar.activation(
                        out=erow[:, nh * 512 : (nh + 1) * 512],
                        in_=psa[:],
                        func=Exp,
                        scale=-SCALE / H,
                    )
                if level >= 2:
                    nc.vector.tensor_scalar_add(out=erow[:], in0=erow[:], scalar1=1.0)
                    nc.vector.reciprocal(out=aff_row[:], in_=erow[:])
                    nc.sync.dma_start(out=of_d[:, :], in_=aff_row[:])
                    # row -> per-n-tile columns via tiny PE outer products
                    afb = stg.tile([1, N], bf16, tag="afb", bufs=1, name="afb")
                    nc.any.tensor_copy(out=afb[:], in_=aff_row[:])
                    for nt in range(NT):
                        psc = ps.tile([P, 1], f32, tag="pj", name="psc")
                        nc.tensor.matmul(
                            psc[:], afb[0:1, nt * P : (nt + 1) * P], ones_row[0:1, 0:1]
                        )
                        nc.vector.tensor_copy(out=aff_cols[:, nt : nt + 1], in_=psc[:])

                # ---- v projection (+ aff scaling into v_aug) ----------
                for nt in range(NT) if level >= 3 else []:
                    psv = ps.tile([P, C], f32, tag="av", bufs=1, name="psv")
                    for ck in range(CK):
                        nc.tensor.matmul(
                            psv[:, 0:512],
                            xv_t[:, ck, nt * P : (nt + 1) * P],
                            wv_t[:, ck, 0:512],
                            start=(ck == 0),
                            stop=(ck == CK - 1),
                        )
                        nc.tensor.matmul(
                            psv[:, 512:C],
                            xv_t[:, ck, nt * P : (nt + 1) * P],
                            wv_t[:, ck, 512:C],
                            start=(ck == 0),
                            stop=(ck == CK - 1),
                        )
                    nc.vector.tensor_scalar_mul(
                        out=v4[:, nt, :, 0:HD],
                        in0=psv[:, :].rearrange("p (h d) -> p h d", d=HD),
                        scalar1=aff_cols[:, nt : nt + 1],
                    )

                # ---- k/q projections interleaved with scores+exp+AV ---
                xq_t = xin.tile([P, CK, N], bf16, tag="x", name="xq_t")
                nc.sync.dma_start(out=xq_t[:], in_=xq_r)
                wq_t = win.tile([P, CK, C], bf16, tag="w", name="wq_t")
                nc.sync.dma_start(out=wq_t[:], in_=wq_r)

                es_tiles = {}

                def emit_scores_pair(pr):
                    """Scores + exp for heads (2*pr, 2*pr+1); row-group packed."""
                    for kt in range(NT):
                        pss = {}
                        for hp in range(2):
                            pss[hp] = ps.tile([P, N], f32, tag="sc", name="pss")
                        for nh in range(2):
                            for hp in range(2):
                                base = hp * HD
                                nc.tensor.matmul(
                                    pss[hp][:, nh * 512 : (nh + 1) * 512],
                                    kT[base : base + HD, pr, kt * P : (kt + 1) * P],
                                    qT[base : base + HD, pr, nh * 512 : (nh + 1) * 512],
                                )
                        for hp in range(2):
                            est = esp.tile([P, N], bf16, tag="es", name="est")
                            nc.scalar.activation(
                                out=est[:], in_=pss[hp][:], func=Exp, scale=SCALE
                            )
                            es_tiles[(2 * pr + hp, kt)] = est

                def emit_av(h):
                    """p@v for head h: out^T rows 0..63, softmax Z in row 64."""
                    ch, hp = divmod(h, 2)
                    psav = ps.tile([HW, N], f32, tag="av", bufs=1, name="psav")
                    for nh in range(2):
                        for kc in range(NT):
                            nc.tensor.matmul(
                                psav[:, nh * 512 : (nh + 1) * 512],
                                v_aug[:, kc, h * HW : (h + 1) * HW],
                                es_tiles[(h, kc)][:, nh * 512 : (nh + 1) * 512],
                                start=(kc == 0),
                                stop=(kc == NT - 1),
                            )
                    zrow = stg.tile([HD + 1, N], f32, tag="zrow", bufs=1, name="zrow")
                    nc.vector.reciprocal(
                        out=zrow[HD : HD + 1, :], in_=psav[HD : HD + 1, :]
                    )
                    # partition_broadcast needs its source at partition 0 on HW;
                    # rebase with a partition-shifting SBUF->SBUF DMA first
                    zrow0 = stg.tile([1, N], f32, tag="zrow0", bufs=1, name="zrow0")
                    nc.sync.dma_start(out=zrow0[:], in_=zrow[HD : HD + 1, :])
                    zb = stg.tile([P, N], f32, tag="zb", bufs=1, name="zb")
                    nc.gpsimd.partition_broadcast(zb[:], zrow0[0:1, :])
                    if hp == 0:
                        nc.vector.tensor_mul(
                            out=xT[0:HD, ch, :], in0=psav[0:HD, :], in1=zb[0:HD, :]
                        )
                    else:
                        xodd = stg.tile([HD, N], bf16, tag="xodd", name="xodd")
                        nc.vector.tensor_mul(
                            out=xodd[:], in0=psav[0:HD, :], in1=zb[0:HD, :]
                        )
                        nc.sync.dma_start(out=xT[HD:P, ch, :], in_=xodd[:])

                for mo in range(CK):
                    for nh in range(2):
                        psk = ps.tile([P, 512], f32, tag="pj", name="psk")
                        for ki in range(CK):
                            nc.tensor.matmul(
                                psk[:],
                                wk_t[:, ki, mo * P : (mo + 1) * P],
                                xk_t[:, ki, nh * 512 : (nh + 1) * 512],
                                start=(ki == 0),
                                stop=(ki == CK - 1),
                            )
                        nc.vector.tensor_copy(
                            out=kT[:, mo, nh * 512 : (nh + 1) * 512], in_=psk[:]
                        )
                    if level < 4:
                        continue
                    for nh in range(2):
                        psq = ps.tile([P, 512], f32, tag="pj", name="psq")
                        for ki in range(CK):
                            nc.tensor.matmul(
                                psq[:],
                                wq_t[:, ki, mo * P : (mo + 1) * P],
                                xq_t[:, ki, nh * 512 : (nh + 1) * 512],
                                start=(ki == 0),
                                stop=(ki == CK - 1),
                            )
                        nc.vector.tensor_copy(
                            out=qT[:, mo, nh * 512 : (nh + 1) * 512], in_=psq[:]
                        )
                    emit_scores_pair(mo)
                    if mo >= 1 and level >= 5:
                        emit_av(2 * (mo - 1))
                        emit_av(2 * (mo - 1) + 1)

                # ---- attn_save (full-C contraction) + remaining AV ----
                def emit_attn_tile(qt):
                    psat = ps.tile([P, N], f32, tag="sc", name="psat")
                    for ck in range(CK):
                        for nh in range(2):
                            nc.tensor.matmul(
                                psat[:, nh * 512 : (nh + 1) * 512],
                                qT[:, ck, qt * P : (qt + 1) * P],
                                kT[:, ck, nh * 512 : (nh + 1) * 512],
                                start=(ck == 0),
                                stop=(ck == CK - 1),
                            )
                    stA = stg.tile([P, N], f32, tag="stA", name="stA")
                    nc.vector.tensor_scalar_mul(
                        out=stA[:], in0=psat[:], scalar1=SCALE / H
                    )
                    nc.sync.dma_start(out=oa_d[qt * P : (qt + 1) * P, :], in_=stA[:])

                if level >= 6:
                    emit_attn_tile(0)
                if level >= 5:
                    emit_av(10)
                if level >= 6:
                    emit_attn_tile(1)
                if level >= 5:
                    emit_av(11)
                if level >= 6:
                    for qt in range(2, NT):
                        emit_attn_tile(qt)

                # ---- output projection --------------------------------
                for qt in range(NT) if level >= 7 else []:
                    psy = ps.tile([P, C], f32, tag="sc", name="psy")
                    for c0, cw in ((0, 512), (512, 256)):
                        for ch in range(CK):
                            nc.tensor.matmul(
                                psy[:, c0 : c0 + cw],
                                xT[:, ch, qt * P : (qt + 1) * P],
                                wpT[:, ch, c0 : c0 + cw],
                                start=(ch == 0),
                                stop=(ch == CK - 1),
                            )
                    ystage = stg.tile([P, C], f32, tag="ystage", name="ystage")
                    nc.vector.tensor_add(out=ystage[:], in0=psy[:], in1=bias_b[:])
                    nc.sync.dma_start(
                        out=ox_d[qt * P : (qt + 1) * P, :], in_=ystage[:]
                    )

    nc.compile()
    return nc


def _get_nc(repeats: int = 1):
    key = repeats
    if key not in _CACHE:
        _CACHE[key] = _build(repeats)
    return _CACHE[key]


def _prep_inputs(xq, xk, xv, extra_token, Wq, Wk, Wv, Wp, bp):
    import ml_dtypes

    bf = ml_dtypes.bfloat16
    arrs = [np.asarray(a, dtype=np.float32) for a in (xq, xk, xv, extra_token)]
    xq, xk, xv, et = arrs
    wq_b = np.ascontiguousarray(np.asarray(Wq, np.float32).T).astype(bf)
    wk_b = np.ascontiguousarray(np.asarray(Wk, np.float32).T).astype(bf)
    wv_b = np.ascontiguousarray(np.asarray(Wv, np.float32).T).astype(bf)
    wp_b = np.ascontiguousarray(np.asarray(Wp, np.float32).T).astype(bf)
    bp_b = np.asarray(bp, np.float32).reshape(1, C).astype(bf)
    in_maps = []
    for b in range(B):
        in_maps.append(
            {
                "xqT": np.ascontiguousarray(xq[b].T).astype(bf),
                "xkT": np.ascontiguousarray(xk[b].T).astype(bf),
                "xvT": np.ascontiguousarray(xv[b].T).astype(bf),
                "wqT": wq_b,
                "wkT": wk_b,
                "wvT": wv_b,
                "wpT": wp_b,
                "uc": np.ascontiguousarray(
                    (np.asarray(Wk, np.float32).T @ et[b]).reshape(CK, P).T
                ).astype(bf),
                "bp": bp_b,
            }
        )
    return in_maps


def kernel(xq, xk, xv, extra_token, Wq, Wk, Wv, Wp, bp):
    from concourse.bass_utils import run_bass_kernel_spmd

    nc = _get_nc(1)
    in_maps = _prep_inputs(xq, xk, xv, extra_token, Wq, Wk, Wv, Wp, bp)
    r = run_bass_kernel_spmd(nc, in_maps, core_ids=list(range(B)))
    x = np.stack([r.results[i]["out_x"] for i in range(B)])
    attn = np.stack([r.results[i]["out_attn"] for i in range(B)])
    aff = np.stack([r.results[i]["out_aff"] for i in range(B)]).reshape(B, 1, 1, N)
    return (
        x.astype(np.float32),
        attn.astype(np.float32),
        aff.astype(np.float32),
    )


if __name__ == "__main__":
    import time

    t0 = time.time()
    nc = _build(1)
    print(f"build+compile(py-side): {time.time() - t0:.1f}s")
    from concourse.timeline_sim import TimelineSim

    t0 = time.time()
    sim = TimelineSim(nc)
    dur = sim.simulate()
    print(f"TimelineSim predicted: {dur:.0f} ns  (sim took {time.time() - t0:.1f}s)")


# revision 15
# speedup vs baseline: 36.3344x; 4.3572x over previous
"""Trainium2 Bass kernel for nn_Attention_70660801954538.

Multi-head attention with an affinity gate and two auxiliary outputs:
    q = xq @ Wq.T ; k = xk @ Wk.T ; v = xv @ Wv.T          (per batch item)
    attn      = (q_h @ k_h.T) * SCALE   per head           [B,H,N,N]
    aff_mask  = sigmoid(mean_h (et_h . k_h) * SCALE)       [B,1,1,N]
    attn_save = mean_h attn                                 [B,N,N]
    x = (aff_mask * softmax(attn)) @ v  -> concat -> @ Wp.T + bp

Sharding: pure data-parallel — one batch item per NeuronCore (B=8, 8 cores),
no collectives.

Key algebraic restructurings (per core):
  * attn_save = (q @ k.T over the FULL C dim) * SCALE/H — one dense matmul
    instead of a 12-way sum of per-head score matrices.
  * aff_mask folds into v (column scaling) instead of scaling the NxN
    probability matrices.
  * softmax denominator Z comes for free from an all-ones column appended to
    each head's v block (row 64 of the AV matmul output).
  * activations/weights are fed to the PE in bf16 (fp32 PSUM accumulation);
    all matmuls then run at the full 1 row/cycle rate.

Layouts: activations are pre-transposed on the host (xT layout [C, N]) so
that every matmul contraction has its index on the partition axis with no
on-chip transposes anywhere.
"""

import numpy as np

B, N, C, H = 8, 1024, 768, 12
HD = C // H           # 64
SCALE = HD ** -0.5    # 0.125
P = 128               # partitions
CK = C // P           # 6 c-chunks of 128
NT = N // P           # 8 n-tiles of 128
HW = HD + 1           # head slot width in v_aug (64 v columns + ones column)

_CACHE: dict = {}


def _build(repeats: int = 1, level: int = 7):
    import concourse.bacc as bacc
    import concourse.mybir as mybir
    from concourse.tile import TileContext

    f32 = mybir.dt.float32
    bf16 = mybir.dt.bfloat16
    Exp = mybir.ActivationFunctionType.Exp

    nc = bacc.Bacc(None, target_bir_lowering=False)

    xqT_d = nc.declare_dram_parameter("xqT", [C, N], bf16, isOutput=False)
    xkT_d = nc.declare_dram_parameter("xkT", [C, N], bf16, isOutput=False)
    xvT_d = nc.declare_dram_parameter("xvT", [C, N], bf16, isOutput=False)
    wqT_d = nc.declare_dram_parameter("wqT", [C, C], bf16, isOutput=False)
    wkT_d = nc.declare_dram_parameter("wkT", [C, C], bf16, isOutput=False)
    wvT_d = nc.declare_dram_parameter("wvT", [C, C], bf16, isOutput=False)
    wpT_d = nc.declare_dram_parameter("wpT", [C, C], bf16, isOutput=False)
    uc_d = nc.declare_dram_parameter("uc", [P, CK], bf16, isOutput=False)
    bp_d = nc.declare_dram_parameter("bp", [1, C], bf16, isOutput=False)
    ox_d = nc.declare_dram_parameter("out_x", [N, C], f32, isOutput=True)
    oa_d = nc.declare_dram_parameter("out_attn", [N, N], f32, isOutput=True)
    of_d = nc.declare_dram_parameter("out_aff", [1, N], f32, isOutput=True)

    xq_r = xqT_d[:, :].rearrange("(ck p) n -> p ck n", p=P)
    xk_r = xkT_d[:, :].rearrange("(ck p) n -> p ck n", p=P)
    xv_r = xvT_d[:, :].rearrange("(ck p) n -> p ck n", p=P)
    wq_r = wqT_d[:, :].rearrange("(ck p) c -> p ck c", p=P)
    wk_r = wkT_d[:, :].rearrange("(ck p) c -> p ck c", p=P)
    wv_r = wvT_d[:, :].rearrange("(ck p) c -> p ck c", p=P)
    wp_r = wpT_d[:, :].rearrange("(ck p) c -> p ck c", p=P)

    with TileContext(nc) as tc:
        with (
            tc.tile_pool(name="per", bufs=1) as per,
            tc.tile_pool(name="stg", bufs=2) as stg,
            tc.tile_pool(name="esp", bufs=22) as esp,
            tc.tile_pool(name="xin", bufs=2) as xin,
            tc.tile_pool(name="win", bufs=2) as win,
            tc.tile_pool(name="ps", bufs=2, space="PSUM") as ps,
        ):
            # ---- persistent tiles -------------------------------------
            qT = per.tile([P, CK, N], bf16)           # q^T  [c, n]
            kT = per.tile([P, CK, N], bf16)           # k^T  [c, n]
            v_aug = per.tile([P, NT, H * HW], bf16)   # [n, head-major v|1]
            xT = per.tile([P, CK, N], bf16)           # attention out^T
            wpT = per.tile([P, CK, C], bf16)
            ones_row = per.tile([P, P], bf16)
            bias_b = per.tile([P, C], f32)            # bp broadcast to rows
            uc = per.tile([P, CK], bf16)              # Wk.T @ extra_token, columns
            bp_r = per.tile([1, C], bf16)
            aff_row = per.tile([1, N], f32)
            aff_cols = per.tile([P, NT], f32)
            erow = per.tile([1, N], f32)

            nc.vector.memset(ones_row[:], 1.0)
            # ones column of every head slot in v_aug
            v4 = v_aug[:, :, :].rearrange("p t (h w) -> p t h w", w=HW)
            nc.vector.memset(v4[:, :, :, HD : HD + 1], 1.0)

            for rep in range(repeats):
                # ---- k projection (inputs first -- on the critical path)
                xk_t = xin.tile([P, CK, N], bf16, tag="x", name="xk_t")
                nc.sync.dma_start(out=xk_t[:], in_=xk_r)
                wk_t = win.tile([P, CK, C], bf16, tag="w", name="wk_t")
                nc.sync.dma_start(out=wk_t[:], in_=wk_r)
                xv_t = xin.tile([P, CK, N], bf16, tag="x", name="xv_t")
                nc.sync.dma_start(out=xv_t[:], in_=xv_r)
                wv_t = win.tile([P, CK, C], bf16, tag="w", name="wv_t")
                nc.sync.dma_start(out=wv_t[:], in_=wv_r)
                xq_t = xin.tile([P, CK, N], bf16, tag="x", name="xq_t")
                nc.sync.dma_start(out=xq_t[:], in_=xq_r)
                wq_t = win.tile([P, CK, C], bf16, tag="w", name="wq_t")
                nc.sync.dma_start(out=wq_t[:], in_=wq_r)

                if rep == 0:
                    # static loads + bias broadcast rows: ones(128,1).T x bp
                    nc.sync.dma_start(out=wpT[:], in_=wp_r)
                    nc.sync.dma_start(out=uc[:], in_=uc_d[:, :])
                    nc.sync.dma_start(out=bp_r[:], in_=bp_d[:, :])
                    psb = ps.tile([P, 512], f32, tag="pj", name="psb")
                    nc.tensor.matmul(psb[:], ones_row[0:1, :], bp_r[:, 0:512])
                    nc.any.tensor_copy(out=bias_b[:, 0:512], in_=psb[:])
                    psb2 = ps.tile([P, 512], f32, tag="pj", name="psb2")
                    nc.tensor.matmul(psb2[:, 0:256], ones_row[0:1, :], bp_r[:, 512:C])
                    nc.any.tensor_copy(out=bias_b[:, 512:C], in_=psb2[:, 0:256])

                # ---- affinity gate row (u = Wk.T et, contracted with xk)
                for nh in range(2) if level >= 2 else []:
                    psa = ps.tile([1, 512], f32, tag="pj", name="psa")
                    for ck in range(CK):
                        nc.tensor.matmul(
                            psa[:],
                            uc[:, ck : ck + 1],
                            xk_t[:, ck, nh * 512 : (nh + 1) * 512],
                            start=(ck == 0),
                            stop=(ck == CK - 1),
                        )
                    # sigmoid(z) = 1/(1+exp(-z)); z = psa * SCALE/H
                    nc.scalar.activation(
                        out=erow[:, nh * 512 : (nh + 1) * 512],
                        in_=psa[:],
                        func=Exp,
                        scale=-SCALE / H,
                    )
                if level >= 2:
                    nc.vector.tensor_scalar_add(out=erow[:], in0=erow[:], scalar1=1.0)
                    nc.vector.reciprocal(out=aff_row[:], in_=erow[:])
                    nc.sync.dma_start(out=of_d[:, :], in_=aff_row[:])
                    # row -> per-n-tile columns via tiny PE outer products
                    afb = stg.tile([1, N], bf16, tag="afb", bufs=1, name="afb")
                    nc.any.tensor_copy(out=afb[:], in_=aff_row[:])
                    for nt in range(NT):
                        psc = ps.tile([P, 1], f32, tag="pj", name="psc")
                        nc.tensor.matmul(
                            psc[:], afb[0:1, nt * P : (nt + 1) * P], ones_row[0:1, 0:1]
                        )
                        nc.vector.tensor_copy(out=aff_cols[:, nt : nt + 1], in_=psc[:])

                # ---- k projection -------------------------------------
                for mo in range(CK):
                    for nh in range(2):
                        psk = ps.tile([P, 512], f32, tag="pj", name="psk")
                        for ki in range(CK):
                            nc.tensor.matmul(
                                psk[:],
                                wk_t[:, ki, mo * P : (mo + 1) * P],
                                xk_t[:, ki, nh * 512 : (nh + 1) * 512],
                                start=(ki == 0),
                                stop=(ki == CK - 1),
                            )
                        nc.vector.tensor_copy(
                            out=kT[:, mo, nh * 512 : (nh + 1) * 512], in_=psk[:]
                        )

                # ---- v projection (+ aff scaling into v_aug) ----------
                for nt in range(NT) if level >= 3 else []:
                    psv = ps.tile([P, C], f32, tag="sc", name="psv")
                    for ck in range(CK):
                        nc.tensor.matmul(
                            psv[:, 0:512],
                            xv_t[:, ck, nt * P : (nt + 1) * P],
                            wv_t[:, ck, 0:512],
                            start=(ck == 0),
                            stop=(ck == CK - 1),
                        )
                        nc.tensor.matmul(
                            psv[:, 512:C],
                            xv_t[:, ck, nt * P : (nt + 1) * P],
                            wv_t[:, ck, 512:C],
                            start=(ck == 0),
                            stop=(ck == CK - 1),
                        )
                    nc.vector.tensor_scalar_mul(
                        out=v4[:, nt, :, 0:HD],
                        in0=psv[:, :].rearrange("p (h d) -> p h d", d=HD),
                        scalar1=aff_cols[:, nt : nt + 1],
                    )

                # ---- q projection interleaved with scores+exp+AV ------
                es_tiles = {}

                def emit_scores_pair(pr):
                    """Scores + exp for heads (2*pr, 2*pr+1); row-group packed."""
                    for kt in range(NT):
                        pss = {}
                        for hp in range(2):
                            pss[hp] = ps.tile([P, N], f32, tag="sc", name="pss")
                        for nh in range(2):
                            for hp in range(2):
                                base = hp * HD
                                nc.tensor.matmul(
                                    pss[hp][:, nh * 512 : (nh + 1) * 512],
                                    kT[base : base + HD, pr, kt * P : (kt + 1) * P],
                                    qT[base : base + HD, pr, nh * 512 : (nh + 1) * 512],
                                )
                        for hp in range(2):
                            est = esp.tile([P, N], bf16, tag="es", name="est")
                            nc.scalar.activation(
                                out=est[:], in_=pss[hp][:], func=Exp, scale=SCALE
                            )
                            es_tiles[(2 * pr + hp, kt)] = est

                def emit_av(h):
                    """p@v for head h: out^T rows 0..63, softmax Z in row 64.

                    Accumulates each q-half into its own single-bank PSUM tile
                    so head h+1's matmuls can start while head h's
                    normalization reads are still draining."""
                    ch, hp = divmod(h, 2)
                    halves = []
                    for nh in range(2):
                        psav = ps.tile([HW, 512], f32, tag="av", name="psav")
                        for kc in range(NT):
                            nc.tensor.matmul(
                                psav[:],
                                v_aug[:, kc, h * HW : (h + 1) * HW],
                                es_tiles[(h, kc)][:, nh * 512 : (nh + 1) * 512],
                                start=(kc == 0),
                                stop=(kc == NT - 1),
                            )
                        halves.append(psav)
                    zrow = stg.tile([HD + 1, N], f32, tag="zrow", bufs=1, name="zrow")
                    for nh in range(2):
                        nc.vector.reciprocal(
                            out=zrow[HD : HD + 1, nh * 512 : (nh + 1) * 512],
                            in_=halves[nh][HD : HD + 1, :],
                        )
                    # partition_broadcast needs its source at partition 0 on HW;
                    # rebase with a partition-shifting SBUF->SBUF DMA first
                    zrow0 = stg.tile([1, N], f32, tag="zrow0", bufs=1, name="zrow0")
                    nc.sync.dma_start(out=zrow0[:], in_=zrow[HD : HD + 1, :])
                    zb = stg.tile([P, N], f32, tag="zb", bufs=2, name="zb")
                    nc.gpsimd.partition_broadcast(zb[:], zrow0[0:1, :])
                    if hp == 0:
                        for nh in range(2):
                            nc.vector.tensor_mul(
                                out=xT[0:HD, ch, nh * 512 : (nh + 1) * 512],
                                in0=halves[nh][0:HD, :],
                                in1=zb[0:HD, nh * 512 : (nh + 1) * 512],
                            )
                    else:
                        xodd = stg.tile([HD, N], bf16, tag="xodd", name="xodd")
                        for nh in range(2):
                            nc.vector.tensor_mul(
                                out=xodd[:, nh * 512 : (nh + 1) * 512],
                                in0=halves[nh][0:HD, :],
                                in1=zb[0:HD, nh * 512 : (nh + 1) * 512],
                            )
                        nc.sync.dma_start(out=xT[HD:P, ch, :], in_=xodd[:])

                for mo in range(CK):
                    if level < 4:
                        continue
                    for nh in range(2):
                        psq = ps.tile([P, 512], f32, tag="pj", name="psq")
                        for ki in range(CK):
                            nc.tensor.matmul(
                                psq[:],
                                wq_t[:, ki, mo * P : (mo + 1) * P],
                                xq_t[:, ki, nh * 512 : (nh + 1) * 512],
                                start=(ki == 0),
                                stop=(ki == CK - 1),
                            )
                        nc.vector.tensor_copy(
                            out=qT[:, mo, nh * 512 : (nh + 1) * 512], in_=psq[:]
                        )
                    emit_scores_pair(mo)
                    if mo >= 1 and level >= 5:
                        emit_av(2 * (mo - 1))
                        emit_av(2 * (mo - 1) + 1)

                # ---- attn_save (full-C contraction) + remaining AV ----
                def emit_attn_tile(qt):
                    psat = ps.tile([P, N], f32, tag="sc", name="psat")
                    for ck in range(CK):
                        for nh in range(2):
                            nc.tensor.matmul(
                                psat[:, nh * 512 : (nh + 1) * 512],
                                qT[:, ck, qt * P : (qt + 1) * P],
                                kT[:, ck, nh * 512 : (nh + 1) * 512],
                                start=(ck == 0),
                                stop=(ck == CK - 1),
                            )
                    stA = stg.tile([P, N], f32, tag="stA", name="stA")
                    nc.vector.tensor_scalar_mul(
                        out=stA[:], in0=psat[:], scalar1=SCALE / H
                    )
                    nc.sync.dma_start(out=oa_d[qt * P : (qt + 1) * P, :], in_=stA[:])

                if level >= 6:
                    emit_attn_tile(0)
                if level >= 5:
                    emit_av(10)
                if level >= 6:
                    emit_attn_tile(1)
                if level >= 5:
                    emit_av(11)
                if level >= 6:
                    for qt in range(2, NT):
                        emit_attn_tile(qt)

                # ---- output projection --------------------------------
                for qt in range(NT) if level >= 7 else []:
                    psy = ps.tile([P, C], f32, tag="sc", name="psy")
                    for c0, cw in ((0, 512), (512, 256)):
                        for ch in range(CK):
                            nc.tensor.matmul(
                                psy[:, c0 : c0 + cw],
                                xT[:, ch, qt * P : (qt + 1) * P],
                                wpT[:, ch, c0 : c0 + cw],
                                start=(ch == 0),
                                stop=(ch == CK - 1),
                            )
                    ystage = stg.tile([P, C], f32, tag="ystage", name="ystage")
                    nc.vector.tensor_add(out=ystage[:], in0=psy[:], in1=bias_b[:])
                    nc.sync.dma_start(
                        out=ox_d[qt * P : (qt + 1) * P, :], in_=ystage[:]
                    )

    nc.compile()
    return nc


def _get_nc(repeats: int = 1):
    key = repeats
    if key not in _CACHE:
        _CACHE[key] = _build(repeats)
    return _CACHE[key]


def _prep_inputs(xq, xk, xv, extra_token, Wq, Wk, Wv, Wp, bp):
    import ml_dtypes

    bf = ml_dtypes.bfloat16
    arrs = [np.asarray(a, dtype=np.float32) for a in (xq, xk, xv, extra_token)]
    xq, xk, xv, et = arrs
    wq_b = np.ascontiguousarray(np.asarray(Wq, np.float32).T).astype(bf)
    wk_b = np.ascontiguousarray(np.asarray(Wk, np.float32).T).astype(bf)
    wv_b = np.ascontiguousarray(np.asarray(Wv, np.float32).T).astype(bf)
    wp_b = np.ascontiguousarray(np.asarray(Wp, np.float32).T).astype(bf)
    bp_b = np.asarray(bp, np.float32).reshape(1, C).astype(bf)
    in_maps = []
    for b in range(B):
        in_maps.append(
            {
                "xqT": np.ascontiguousarray(xq[b].T).astype(bf),
                "xkT": np.ascontiguousarray(xk[b].T).astype(bf),
                "xvT": np.ascontiguousarray(xv[b].T).astype(bf),
                "wqT": wq_b,
                "wkT": wk_b,
                "wvT": wv_b,
                "wpT": wp_b,
                "uc": np.ascontiguousarray(
                    (np.asarray(Wk, np.float32).T @ et[b]).reshape(CK, P).T
                ).astype(bf),
                "bp": bp_b,
            }
        )
    return in_maps


def kernel(xq, xk, xv, extra_token, Wq, Wk, Wv, Wp, bp):
    from concourse.bass_utils import run_bass_kernel_spmd

    nc = _get_nc(1)
    in_maps = _prep_inputs(xq, xk, xv, extra_token, Wq, Wk, Wv, Wp, bp)
    r = run_bass_kernel_spmd(nc, in_maps, core_ids=list(range(B)))
    x = np.stack([r.results[i]["out_x"] for i in range(B)])
    attn = np.stack([r.results[i]["out_attn"] for i in range(B)])
    aff = np.stack([r.results[i]["out_aff"] for i in range(B)]).reshape(B, 1, 1, N)
    return (
        x.astype(np.float32),
        attn.astype(np.float32),
        aff.astype(np.float32),
    )


if __name__ == "__main__":
    import time

    t0 = time.time()
    nc = _build(1)
    print(f"build+compile(py-side): {time.time() - t0:.1f}s")
    from concourse.timeline_sim import TimelineSim

    t0 = time.time()
    sim = TimelineSim(nc)
    dur = sim.simulate()
    print(f"TimelineSim predicted: {dur:.0f} ns  (sim took {time.time() - t0:.1f}s)")
